# revision 17
# baseline (speedup 1.0000x reference)
"""Trainium2 Bass kernel for batched multi-head attention with per-head
clamped-exp temperature (nn_Attention_91173565760008).

  reference:
    scale = exp(min(logit_scale, ln(100)))          # [H,1,1]
    dots  = einsum('bhnd,bhmd->bhnm', q, k) * scale
    attn  = softmax(dots, -1)
    out   = einsum('bhnm,bhmd->bhnd', attn, v)

Shapes: B=4, H=12, N=2048, D=64, fp32.  8 NeuronCores, (B*H)=48 head-pairs
sharded 6 per core (data + head parallel, per the sharding hint).

Transpose-free design.  The baseline computed P = softmax rows in natural
layout and moved P^T through the DMA xbar (50MB/core of SBUF<->SBUF traffic,
~58% busy on all 16 queues).  Instead this kernel computes S twice on the
underutilized PE and never transposes P:

  per (b,h) pair:
    Snat[q,m] = (scale q)^T k        fp32r PE, 512-col chunks     [rowmax only]
    negmax[q] = -rowmax(Snat)        DVE tensor_tensor_reduce (fused
                                     elementwise-max + min-reduce, 2 per tile)
    row 64 of qt_aug <- negmax       DVE 32x32 stream transpose + tiny DMAs
    S^T[m,q]  = kt_aug^T qt_aug      K=65 matmul: row 64 = (ones)x(-max),
                                     i.e. the max subtraction is folded into
                                     the contraction -> bias-free exp
    P^T       = exp(S^T)             ACT -> fp16 SBUF, natural for PV
    O^T[d,q] += V_aug_t^T P^T_t      fp16 PE, fp32 PSUM accum over all 16
                                     m-tiles; V has a ones column so row 64
                                     of O^T is the softmax denominator
  host: out = O^T[0:64]/O^T[64] transposed back.  No online-softmax merge.
"""

import sys

sys.path.insert(0, "/opt/trn_rl_repo")

import numpy as np

B, H, N, D = 4, 12, 2048, 64
NCORES = 8
PAIRS = (B * H) // NCORES  # 6 head-pairs per core
QTILES = 16  # 128-row query tiles per pair
MTILES = 16  # 128-row key tiles per pair
NH = 2  # query superchunks (1024 cols each)
CH = N // NH  # 1024
DA = D + 1  # 65: head dim + bias/ones row
MAX_LOG_SCALE = 4.6052  # ln(100) clamp from the module

_CACHE = {}


def _build_nc(reps=1):
    """Build + compile the single-core Tile program (same program runs SPMD
    on all 8 cores with different data).  reps>1 repeats the whole
    computation (same I/O) -- used only to measure marginal kernel time."""
    from contextlib import ExitStack

    import concourse.bacc as bacc
    import concourse.tile as tile
    from concourse import mybir

    f32 = mybir.dt.float32
    f32r = mybir.dt.float32r
    f16 = mybir.dt.float16
    MAX = mybir.AluOpType.max
    MIN = mybir.AluOpType.min
    EXP = mybir.ActivationFunctionType.Exp

    nc = bacc.Bacc("TRN2", target_bir_lowering=False, debug=False)

    qt_d = nc.dram_tensor("qt", [PAIRS, 128, N], f32r, kind="ExternalInput")
    kt_d = nc.dram_tensor("kt", [PAIRS, 128, N], f32r, kind="ExternalInput")
    vt_d = nc.dram_tensor("vt", [PAIRS, 128, MTILES, DA], f16, kind="ExternalInput")
    ot_d = nc.dram_tensor("ot", [PAIRS, DA, N], f32, kind="ExternalOutput")

    with ExitStack() as ctx:
        tc = ctx.enter_context(tile.TileContext(nc))
        io_pool = ctx.enter_context(tc.tile_pool(name="io", bufs=2))
        ob_pool = ctx.enter_context(tc.tile_pool(name="ob", bufs=2))
        pt_pool = ctx.enter_context(tc.tile_pool(name="pt", bufs=8))
        sc_pool = ctx.enter_context(tc.tile_pool(name="sc", bufs=2))
        # PSUM budget (8 banks x 2KB): st 3x[128,512] = 3, sn 3x[128,512] = 3,
        # ov 1x[65,1024] = 2
        ps_s = ctx.enter_context(tc.tile_pool(name="ps_s", bufs=3, space="PSUM"))
        ps_n = ctx.enter_context(tc.tile_pool(name="ps_n", bufs=3, space="PSUM"))
        ps_o = ctx.enter_context(tc.tile_pool(name="ps_o", bufs=1, space="PSUM"))

        seq = [(r, p) for r in range(reps) for p in range(PAIRS)]

        def load_pair(idx):
            r, p = seq[idx]
            # chunked loads: the first Snat matmuls only need the first
            # 512-col pieces, so don't gate them on whole-tile transfers
            qa = io_pool.tile([128, N], f32r, tag="qa", name=f"qa{r}_{p}")
            ka = io_pool.tile([128, N], f32r, tag="ka", name=f"ka{r}_{p}")
            for c in range(4):
                sl = slice(512 * c, 512 * (c + 1))
                nc.gpsimd.dma_start(ka[:, sl], kt_d[p, :, sl])
                nc.gpsimd.dma_start(qa[:, sl], qt_d[p, :, sl])
            va = io_pool.tile([128, MTILES, DA], f16, tag="va", name=f"va{r}_{p}")
            nc.gpsimd.dma_start(va[:], vt_d[p])
            nm = sc_pool.tile([128, 32], f32, tag="nm", name=f"nm{r}_{p}")
            nc.gpsimd.memset(nm[:], 0.0)
            return qa, ka, va, nm

        def snat_phase(st8, i, phase):
            """Half a q-tile's rowmax work: 2 fp32r matmuls over a 1024-col
            m-range into two single-bank PSUM tiles + 2 per-chunk DVE
            reduces.  phase 1 merges the 4 partials into nm[:, i] (negated).
            DVE may read only ONE PSUM operand per instruction, and the
            2-buffer sn ring keeps the PE from ever waiting on a reduce."""
            qa, ka, va, nm = st8
            if phase == 0:
                pm = sc_pool.tile([128, 4], f32, tag="pm", name=f"pm{i}")
                _pm_live[0] = pm
            else:
                pm = _pm_live[0]
            for c in range(2):
                mo = 1024 * phase + 512 * c
                sn = ps_n.tile([128, 512], f32, tag="sn", name=f"sn{i}_{phase}_{c}")
                nc.tensor.matmul(
                    sn[:],
                    lhsT=qa[:, 128 * i : 128 * (i + 1)],
                    rhs=ka[:, mo : mo + 512],
                    start=True,
                    stop=True,
                )
                nc.vector.tensor_reduce(
                    pm[:, 2 * phase + c : 2 * phase + c + 1],
                    sn[:],
                    axis=mybir.AxisListType.X,
                    op=MAX,
                )
            if phase == 1:
                nc.vector.tensor_reduce(
                    nm[:, i : i + 1],
                    pm[:],
                    axis=mybir.AxisListType.X,
                    op=MAX,
                    negate=True,
                )

        _pm_live = [None]

        def negmax_row(qa, nm, h):
            """Scatter negmax for q-tiles 8h..8h+7 into qa row 64 (the bias
            row) via a 32x32 block transpose + 4 small strided DMAs (on the
            sync/scalar HWDGE queues -- much lower latency than SWDGE)."""
            nmT = sc_pool.tile([128, 32], f32, tag="nmT", name=f"nmT{h}")
            nc.vector.transpose(nmT[:], nm[:])
            row = qa[64:65, CH * h : CH * (h + 1)].rearrange(
                "p (u j s) -> p u j s", u=8, j=4, s=32
            )
            for j in range(4):
                eng = nc.sync
                eng.dma_start(
                    row[:, :, j],
                    nmT[32 * j + 8 * h : 32 * j + 8 * h + 8, 0:32].bitcast(f32r),
                )

        nxt = load_pair(0)
        # prologue: pair 0, q-tiles 0-7 rowmax
        for i in range(8):
            snat_phase(nxt, i, 0)
            snat_phase(nxt, i, 1)

        for idx in range(len(seq)):
            _, p = seq[idx]
            cur = nxt
            qa, ka, va, nm = cur
            ot_sb = ob_pool.tile([DA, N], f32, tag="ot", name=f"ot{idx}")

            if idx + 1 < len(seq):
                nxt = load_pair(idx + 1)

            for h in range(NH):
                negmax_row(qa, nm, h)
                if h == 0:
                    snat_phase(cur, 8, 0)
                elif idx + 1 < len(seq):
                    snat_phase(nxt, 0, 0)
                ov = ps_o.tile([DA, 1024], f32, tag="ov", name=f"ov{idx}_{h}")
                pts = [None] * MTILES
                for t in range(MTILES + 3):
                    if t >= 3:
                        # software-pipelined three m-tiles behind the S^T
                        # matmuls so the PE never waits on the ACT exp
                        pva, pvb = pts[t - 3]
                        nc.tensor.matmul(
                            ov[:, 0:512],
                            lhsT=va[:, t - 3, :],
                            rhs=pva[:],
                            start=(t == 3),
                            stop=(t == MTILES + 2),
                        )
                        nc.tensor.matmul(
                            ov[:, 512:1024],
                            lhsT=va[:, t - 3, :],
                            rhs=pvb[:],
                            start=(t == 3),
                            stop=(t == MTILES + 2),
                        )
                    if t < MTILES:
                        sta = ps_s.tile([128, 512], f32, tag="st", name=f"sta{t}")
                        nc.tensor.matmul(
                            sta[:],
                            lhsT=ka[:, 128 * t : 128 * (t + 1)],
                            rhs=qa[:, CH * h : CH * h + 512],
                            start=True,
                            stop=True,
                        )
                        stb = ps_s.tile([128, 512], f32, tag="st", name=f"stb{t}")
                        nc.tensor.matmul(
                            stb[:],
                            lhsT=ka[:, 128 * t : 128 * (t + 1)],
                            rhs=qa[:, CH * h + 512 : CH * h + 1024],
                            start=True,
                            stop=True,
                        )
                        pta = pt_pool.tile([128, 512], f16, tag="pt", name=f"pta{t}")
                        nc.scalar.activation(pta[:], sta[:], EXP)
                        ptb = pt_pool.tile([128, 512], f16, tag="pt", name=f"ptb{t}")
                        nc.scalar.activation(ptb[:], stb[:], EXP)
                        pts[t] = (pta, ptb)
                    # weave the next half/pair's rowmax work between slots
                    # (shifted one slot early; slot 15 is free so the bias
                    # transpose+DMA latency hides before the boundary)
                    if t + 1 < MTILES:
                        ph = t + 1
                        if h == 0:
                            snat_phase(cur, 8 + ph // 2, ph % 2)
                        elif idx + 1 < len(seq):
                            snat_phase(nxt, ph // 2, ph % 2)
                # drain the PV accumulator and ship this half immediately
                nc.scalar.copy(ot_sb[:, CH * h : CH * (h + 1)], ov[:])
                nc.gpsimd.dma_start(
                    ot_d[p, :, CH * h : CH * (h + 1)],
                    ot_sb[:, CH * h : CH * (h + 1)],
                )

    nc.compile()
    return nc


def _get_nc(reps=1):
    key = f"nc{reps}"
    if key not in _CACHE:
        _CACHE[key] = _build_nc(reps)
    return _CACHE[key]


def _prep_inputs(q, k, v, logit_scale):
    """Host-side marshalling: scale fold, transposes, aug rows, sharding."""
    scale = np.exp(
        np.minimum(logit_scale.astype(np.float32), np.float32(MAX_LOG_SCALE))
    ).reshape(H)
    qs = q.astype(np.float32) * scale[None, :, None, None].astype(np.float32)

    # rows 0-63: data; row 64: bias (q: 0 placeholder / k: ones); 65-127: zero
    qT = np.zeros((B * H, 128, N), np.float32)
    qT[:, :D] = qs.transpose(0, 1, 3, 2).reshape(B * H, D, N)
    kT = np.zeros((B * H, 128, N), np.float32)
    kT[:, :D] = k.astype(np.float32).transpose(0, 1, 3, 2).reshape(B * H, D, N)
    kT[:, D] = 1.0
    # vt[bh, p, t, d] = v[bh, 128 t + p, d], plus a ones column at d=64
    vt = np.ones((B * H, 128, MTILES, DA), np.float16)
    vt[:, :, :, :D] = (
        v.astype(np.float32)
        .reshape(B, H, MTILES, 128, D)
        .transpose(0, 1, 3, 2, 4)
        .reshape(B * H, 128, MTILES, D)
        .astype(np.float16)
    )
    in_maps = []
    for c in range(NCORES):
        sl = slice(PAIRS * c, PAIRS * (c + 1))
        in_maps.append(
            {
                "qt": np.ascontiguousarray(qT[sl]),
                "kt": np.ascontiguousarray(kT[sl]),
                "vt": np.ascontiguousarray(vt[sl]),
            }
        )
    return in_maps


def _assemble(results):
    """out[q, d] = O^T[d, q] / O^T[64, q], transposed back -> [B,H,N,D]."""
    out = np.empty((B * H, N, D), np.float32)
    for c in range(NCORES):
        ot = results[c]["ot"]  # [PAIRS, DA, N]
        for p in range(PAIRS):
            bh = PAIRS * c + p
            out[bh] = (ot[p, :D] / ot[p, D : D + 1]).T
    return out.reshape(B, H, N, D)


def kernel(q, k, v, logit_scale):
    from concourse.bass_utils import run_bass_kernel_spmd

    in_maps = _prep_inputs(q, k, v, logit_scale)
    nc = _get_nc()
    res = run_bass_kernel_spmd(nc, in_maps, list(range(NCORES)))
    return _assemble(res.results)


# revision 20
# speedup vs baseline: 1.2002x; 1.2002x over previous
"""Trainium2 Bass kernel for batched multi-head attention with per-head
clamped-exp temperature (nn_Attention_91173565760008).

  reference:
    scale = exp(min(logit_scale, ln(100)))          # [H,1,1]
    dots  = einsum('bhnd,bhmd->bhnm', q, k) * scale
    attn  = softmax(dots, -1)
    out   = einsum('bhnm,bhmd->bhnd', attn, v)

Shapes: B=4, H=12, N=2048, D=64, fp32.  8 NeuronCores, (B*H)=48 head-pairs
sharded 6 per core (data + head parallel, per the sharding hint).

Transpose-free attention.  The m-contraction axis of PV wants P transposed
relative to the softmax layout; rather than moving N^2 fp16 through the DMA
xbar (the old baseline: ~58% busy on all 16 queues), this kernel computes S
twice on the PE and never transposes P:

  per (b,h) pair:
    Snat[q,m] = (scale q)^T k      fp32r, K zero-padded to 128 (K in the
                                   64..80 range runs at HALF rate on TRN2;
                                   K>=96 streams 1 col/cycle, measured)
    negmax[q] = -rowmax(Snat)      DVE per-chunk PSUM reduces + tiny merge
                                   (DVE allows only one PSUM operand/inst)
    qt row 64 <- negmax            DVE 32x32 stream-transpose + 4 strided
                                   HWDGE DMAs into the q operand's bias row
    S^T[m,q]  = kt_aug^T qt_aug    K=128 fp32r: row 64 = ones x (-max), so
                                   the max subtraction rides the contraction
                                   and the exp needs no bias
    P^T       = exp(S^T)           ACT -> fp16 SBUF, already PV-ready
    O^T[d,q] += V_aug_t^T P^T_t    fp16, fp32 PSUM accum over 16 m-tiles;
                                   V's ones column makes row 64 of O^T the
                                   softmax denominator
  host: out = O^T[0:64]/O^T[64], transposed back.  No softmax merge pass.

Schedule: per superchunk (1024 q cols) the PE runs 16 m-tile slots of
[2 PV (lagged 3 slots behind their exp), 2 S^T, 2 Snat-for-the-next-half]
so all five engines pipeline; PSUM is exactly 8 banks (st 3x[128,512],
sn 3x[128,512], ov [128,1024]).  Measured ~320 us vs 652 us baseline.
"""

import sys

sys.path.insert(0, "/opt/trn_rl_repo")

import numpy as np

B, H, N, D = 4, 12, 2048, 64
NCORES = 8
PAIRS = (B * H) // NCORES  # 6 head-pairs per core
QTILES = 16  # 128-row query tiles per pair
MTILES = 16  # 128-row key tiles per pair
NH = 2  # query superchunks (1024 cols each)
CH = N // NH  # 1024
DA = D + 1  # 65: head dim + bias/ones row
MAX_LOG_SCALE = 4.6052  # ln(100) clamp from the module

_CACHE = {}


def _build_nc(reps=1):
    """Build + compile the single-core Tile program (same program runs SPMD
    on all 8 cores with different data).  reps>1 repeats the whole
    computation (same I/O) -- used only to measure marginal kernel time."""
    from contextlib import ExitStack

    import concourse.bacc as bacc
    import concourse.tile as tile
    from concourse import mybir

    f32 = mybir.dt.float32
    f32r = mybir.dt.float32r
    f16 = mybir.dt.float16
    MAX = mybir.AluOpType.max
    MIN = mybir.AluOpType.min
    EXP = mybir.ActivationFunctionType.Exp

    nc = bacc.Bacc("TRN2", target_bir_lowering=False, debug=False)

    qt_d = nc.dram_tensor("qt", [PAIRS, 128, N], f32r, kind="ExternalInput")
    kt_d = nc.dram_tensor("kt", [PAIRS, 128, N], f32r, kind="ExternalInput")
    vt_d = nc.dram_tensor("vt", [PAIRS, 128, MTILES, DA], f16, kind="ExternalInput")
    ot_d = nc.dram_tensor("ot", [PAIRS, DA, N], f32, kind="ExternalOutput")

    with ExitStack() as ctx:
        tc = ctx.enter_context(tile.TileContext(nc))
        io_pool = ctx.enter_context(tc.tile_pool(name="io", bufs=2))
        ob_pool = ctx.enter_context(tc.tile_pool(name="ob", bufs=2))
        pt_pool = ctx.enter_context(tc.tile_pool(name="pt", bufs=8))
        sc_pool = ctx.enter_context(tc.tile_pool(name="sc", bufs=2))
        # PSUM budget (8 banks x 2KB): st 3x[128,512] = 3, sn 3x[128,512] = 3,
        # ov 1x[65,1024] = 2
        ps_s = ctx.enter_context(tc.tile_pool(name="ps_s", bufs=3, space="PSUM"))
        ps_n = ctx.enter_context(tc.tile_pool(name="ps_n", bufs=3, space="PSUM"))
        ps_o = ctx.enter_context(tc.tile_pool(name="ps_o", bufs=1, space="PSUM"))

        seq = [(r, p) for r in range(reps) for p in range(PAIRS)]

        def load_pair(idx):
            r, p = seq[idx]
            # chunked loads: the first Snat matmuls only need the first
            # 512-col pieces, so don't gate them on whole-tile transfers
            qa = io_pool.tile([128, N], f32r, tag="qa", name=f"qa{r}_{p}")
            ka = io_pool.tile([128, N], f32r, tag="ka", name=f"ka{r}_{p}")
            for c in range(4):
                sl = slice(512 * c, 512 * (c + 1))
                nc.gpsimd.dma_start(ka[:, sl], kt_d[p, :, sl])
                nc.gpsimd.dma_start(qa[:, sl], qt_d[p, :, sl])
            va = io_pool.tile([128, MTILES, DA], f16, tag="va", name=f"va{r}_{p}")
            nc.gpsimd.dma_start(va[:], vt_d[p])
            nm = sc_pool.tile([128, 32], f32, tag="nm", name=f"nm{r}_{p}")
            nc.gpsimd.memset(nm[:], 0.0)
            return qa, ka, va, nm

        def snat_phase(st8, i, phase):
            """Half a q-tile's rowmax work: 2 fp32r matmuls over a 1024-col
            m-range into two single-bank PSUM tiles + 2 per-chunk DVE
            reduces.  phase 1 merges the 4 partials into nm[:, i] (negated).
            DVE may read only ONE PSUM operand per instruction, and the
            2-buffer sn ring keeps the PE from ever waiting on a reduce."""
            qa, ka, va, nm = st8
            if phase == 0:
                pm = sc_pool.tile([128, 4], f32, tag="pm", name=f"pm{i}")
                _pm_live[0] = pm
            else:
                pm = _pm_live[0]
            for c in range(2):
                mo = 1024 * phase + 512 * c
                sn = ps_n.tile([128, 512], f32, tag="sn", name=f"sn{i}_{phase}_{c}")
                nc.tensor.matmul(
                    sn[:],
                    lhsT=qa[:, 128 * i : 128 * (i + 1)],
                    rhs=ka[:, mo : mo + 512],
                    start=True,
                    stop=True,
                )
                nc.vector.tensor_reduce(
                    pm[:, 2 * phase + c : 2 * phase + c + 1],
                    sn[:],
                    axis=mybir.AxisListType.X,
                    op=MAX,
                )
            if phase == 1:
                nc.vector.tensor_reduce(
                    nm[:, i : i + 1],
                    pm[:],
                    axis=mybir.AxisListType.X,
                    op=MAX,
                    negate=True,
                )

        _pm_live = [None]

        def negmax_row(qa, nm, h):
            """Scatter negmax for q-tiles 8h..8h+7 into qa row 64 (the bias
            row) via a 32x32 block transpose + 4 small strided DMAs (on the
            sync/scalar HWDGE queues -- much lower latency than SWDGE)."""
            nmT = sc_pool.tile([128, 32], f32, tag="nmT", name=f"nmT{h}")
            nc.vector.transpose(nmT[:], nm[:])
            row = qa[64:65, CH * h : CH * (h + 1)].rearrange(
                "p (u j s) -> p u j s", u=8, j=4, s=32
            )
            for j in range(4):
                eng = nc.sync
                eng.dma_start(
                    row[:, :, j],
                    nmT[32 * j + 8 * h : 32 * j + 8 * h + 8, 0:32].bitcast(f32r),
                )

        nxt = load_pair(0)
        # prologue: pair 0, q-tiles 0-7 rowmax
        for i in range(8):
            snat_phase(nxt, i, 0)
            snat_phase(nxt, i, 1)

        for idx in range(len(seq)):
            _, p = seq[idx]
            cur = nxt
            qa, ka, va, nm = cur
            ot_sb = ob_pool.tile([DA, N], f32, tag="ot", name=f"ot{idx}")

            if idx + 1 < len(seq):
                nxt = load_pair(idx + 1)

            for h in range(NH):
                negmax_row(qa, nm, h)
                if h == 0:
                    snat_phase(cur, 8, 0)
                elif idx + 1 < len(seq):
                    snat_phase(nxt, 0, 0)
                ov = ps_o.tile([DA, 1024], f32, tag="ov", name=f"ov{idx}_{h}")
                pts = [None] * MTILES
                for t in range(MTILES + 3):
                    if t >= 3:
                        # software-pipelined three m-tiles behind the S^T
                        # matmuls so the PE never waits on the ACT exp
                        pva, pvb = pts[t - 3]
                        nc.tensor.matmul(
                            ov[:, 0:512],
                            lhsT=va[:, t - 3, :],
                            rhs=pva[:],
                            start=(t == 3),
                            stop=(t == MTILES + 2),
                        )
                        nc.tensor.matmul(
                            ov[:, 512:1024],
                            lhsT=va[:, t - 3, :],
                            rhs=pvb[:],
                            start=(t == 3),
                            stop=(t == MTILES + 2),
                        )
                    if t < MTILES:
                        sta = ps_s.tile([128, 512], f32, tag="st", name=f"sta{t}")
                        nc.tensor.matmul(
                            sta[:],
                            lhsT=ka[:, 128 * t : 128 * (t + 1)],
                            rhs=qa[:, CH * h : CH * h + 512],
                            start=True,
                            stop=True,
                        )
                        stb = ps_s.tile([128, 512], f32, tag="st", name=f"stb{t}")
                        nc.tensor.matmul(
                            stb[:],
                            lhsT=ka[:, 128 * t : 128 * (t + 1)],
                            rhs=qa[:, CH * h + 512 : CH * h + 1024],
                            start=True,
                            stop=True,
                        )
                        pta = pt_pool.tile([128, 512], f16, tag="pt", name=f"pta{t}")
                        nc.scalar.activation(pta[:], sta[:], EXP)
                        ptb = pt_pool.tile([128, 512], f16, tag="pt", name=f"ptb{t}")
                        nc.scalar.activation(ptb[:], stb[:], EXP)
                        pts[t] = (pta, ptb)
                    # weave the next half/pair's rowmax work between slots
                    # (shifted one slot early; slot 15 is free so the bias
                    # transpose+DMA latency hides before the boundary)
                    if t + 1 < MTILES:
                        ph = t + 1
                        if h == 0:
                            snat_phase(cur, 8 + ph // 2, ph % 2)
                        elif idx + 1 < len(seq):
                            snat_phase(nxt, ph // 2, ph % 2)
                # drain the PV accumulator and ship this half immediately
                nc.scalar.copy(ot_sb[:, CH * h : CH * (h + 1)], ov[:])
                nc.gpsimd.dma_start(
                    ot_d[p, :, CH * h : CH * (h + 1)],
                    ot_sb[:, CH * h : CH * (h + 1)],
                )

    nc.compile()
    return nc


def _get_nc(reps=1):
    key = f"nc{reps}"
    if key not in _CACHE:
        _CACHE[key] = _build_nc(reps)
    return _CACHE[key]


def _prep_inputs(q, k, v, logit_scale):
    """Host-side marshalling: scale fold, transposes, aug rows, sharding."""
    scale = np.exp(
        np.minimum(logit_scale.astype(np.float32), np.float32(MAX_LOG_SCALE))
    ).reshape(H)
    qs = q.astype(np.float32) * scale[None, :, None, None].astype(np.float32)

    # rows 0-63: data; row 64: bias (q: 0 placeholder / k: ones); 65-127: zero
    qT = np.zeros((B * H, 128, N), np.float32)
    qT[:, :D] = qs.transpose(0, 1, 3, 2).reshape(B * H, D, N)
    kT = np.zeros((B * H, 128, N), np.float32)
    kT[:, :D] = k.astype(np.float32).transpose(0, 1, 3, 2).reshape(B * H, D, N)
    kT[:, D] = 1.0
    # vt[bh, p, t, d] = v[bh, 128 t + p, d], plus a ones column at d=64
    vt = np.ones((B * H, 128, MTILES, DA), np.float16)
    vt[:, :, :, :D] = (
        v.astype(np.float32)
        .reshape(B, H, MTILES, 128, D)
        .transpose(0, 1, 3, 2, 4)
        .reshape(B * H, 128, MTILES, D)
        .astype(np.float16)
    )
    in_maps = []
    for c in range(NCORES):
        sl = slice(PAIRS * c, PAIRS * (c + 1))
        in_maps.append(
            {
                "qt": np.ascontiguousarray(qT[sl]),
                "kt": np.ascontiguousarray(kT[sl]),
                "vt": np.ascontiguousarray(vt[sl]),
            }
        )
    return in_maps


def _assemble(results):
    """out[q, d] = O^T[d, q] / O^T[64, q], transposed back -> [B,H,N,D]."""
    out = np.empty((B * H, N, D), np.float32)
    for c in range(NCORES):
        ot = results[c]["ot"]  # [PAIRS, DA, N]
        for p in range(PAIRS):
            bh = PAIRS * c + p
            out[bh] = (ot[p, :D] / ot[p, D : D + 1]).T
    return out.reshape(B, H, N, D)


def kernel(q, k, v, logit_scale):
    from concourse.bass_utils import run_bass_kernel_spmd

    in_maps = _prep_inputs(q, k, v, logit_scale)
    nc = _get_nc()
    res = run_bass_kernel_spmd(nc, in_maps, list(range(NCORES)))
    return _assemble(res.results)


# revision 26
# speedup vs baseline: 1.2012x; 1.0008x over previous
"""Trainium2 Bass kernel for batched multi-head attention with per-head
clamped-exp temperature (nn_Attention_91173565760008).

  reference:
    scale = exp(min(logit_scale, ln(100)))          # [H,1,1]
    dots  = einsum('bhnd,bhmd->bhnm', q, k) * scale
    attn  = softmax(dots, -1)
    out   = einsum('bhnm,bhmd->bhnd', attn, v)

Shapes: B=4, H=12, N=2048, D=64, fp32.  8 NeuronCores, (B*H)=48 head-pairs
sharded 6 per core (data + head parallel, per the sharding hint).

Transpose-free attention.  The m-contraction axis of PV wants P transposed
relative to the softmax layout; rather than moving N^2 fp16 through the DMA
xbar (the old baseline: ~58% busy on all 16 queues), this kernel computes S
twice on the PE and never transposes P:

  per (b,h) pair:
    Snat[q,m] = (scale q)^T k      fp32r, K zero-padded to 128 (K in the
                                   64..80 range runs at HALF rate on TRN2;
                                   K>=96 streams 1 col/cycle, measured)
    negmax[q] = -rowmax(Snat)      DVE per-chunk PSUM reduces + tiny merge
                                   (DVE allows only one PSUM operand/inst)
    qt row 64 <- negmax            DVE 32x32 stream-transpose + 4 strided
                                   HWDGE DMAs into the q operand's bias row
    S^T[m,q]  = kt_aug^T qt_aug    K=128 fp32r: row 64 = ones x (-max), so
                                   the max subtraction rides the contraction
                                   and the exp needs no bias
    P^T       = exp(S^T)           ACT -> fp16 SBUF, already PV-ready
    O^T[d,q] += V_aug_t^T P^T_t    fp16, fp32 PSUM accum over 16 m-tiles;
                                   V's ones column makes row 64 of O^T the
                                   softmax denominator
  host: out = O^T[0:64]/O^T[64], transposed back.  No softmax merge pass.

Schedule: each superchunk (1024 q cols) runs as two overlapped 512-col
passes offset by 16 slots sharing one ov PSUM bank; each slot issues
[1 PV (lagged 3 slots behind its exp), 1 S^T matmul, exp] with the next
half's rowmax matmuls woven between slots, so all five engines pipeline.
PSUM is exactly 8 banks (st 3x[128,512], sn 2x[128,1024], ov [65,512]).
Measured ~320 us vs 652 us baseline (PE ~90% busy, back-to-back).
"""

import sys

sys.path.insert(0, "/opt/trn_rl_repo")

import numpy as np

B, H, N, D = 4, 12, 2048, 64
NCORES = 8
PAIRS = (B * H) // NCORES  # 6 head-pairs per core
QTILES = 16  # 128-row query tiles per pair
MTILES = 16  # 128-row key tiles per pair
NH = 2  # query superchunks (1024 cols each)
CH = N // NH  # 1024
DA = D + 1  # 65: head dim + bias/ones row
MAX_LOG_SCALE = 4.6052  # ln(100) clamp from the module

_CACHE = {}


def _build_nc(reps=1):
    """Build + compile the single-core Tile program (same program runs SPMD
    on all 8 cores with different data).  reps>1 repeats the whole
    computation (same I/O) -- used only to measure marginal kernel time."""
    from contextlib import ExitStack

    import concourse.bacc as bacc
    import concourse.tile as tile
    from concourse import mybir

    f32 = mybir.dt.float32
    f32r = mybir.dt.float32r
    f16 = mybir.dt.float16
    MAX = mybir.AluOpType.max
    MIN = mybir.AluOpType.min
    EXP = mybir.ActivationFunctionType.Exp

    nc = bacc.Bacc("TRN2", target_bir_lowering=False, debug=False)

    qt_d = nc.dram_tensor("qt", [PAIRS, 128, N], f32r, kind="ExternalInput")
    kt_d = nc.dram_tensor("kt", [PAIRS, 128, N], f32r, kind="ExternalInput")
    vt_d = nc.dram_tensor("vt", [PAIRS, 128, MTILES, DA], f16, kind="ExternalInput")
    ot_d = nc.dram_tensor("ot", [PAIRS, DA, N], f32, kind="ExternalOutput")

    with ExitStack() as ctx:
        tc = ctx.enter_context(tile.TileContext(nc))
        io_pool = ctx.enter_context(tc.tile_pool(name="io", bufs=2))
        ob_pool = ctx.enter_context(tc.tile_pool(name="ob", bufs=2))
        pt_pool = ctx.enter_context(tc.tile_pool(name="pt", bufs=8))
        sc_pool = ctx.enter_context(tc.tile_pool(name="sc", bufs=2))
        # PSUM budget (8 banks x 2KB): st 3x[128,512] = 3, sn 2x[128,1024] = 4,
        # ov 1x[65,512] = 1
        ps_s = ctx.enter_context(tc.tile_pool(name="ps_s", bufs=3, space="PSUM"))
        ps_n = ctx.enter_context(tc.tile_pool(name="ps_n", bufs=2, space="PSUM"))
        ps_o = ctx.enter_context(tc.tile_pool(name="ps_o", bufs=1, space="PSUM"))

        seq = [(r, p) for r in range(reps) for p in range(PAIRS)]

        def load_pair(idx):
            r, p = seq[idx]
            # chunked loads: the first Snat matmuls only need the first
            # 512-col pieces, so don't gate them on whole-tile transfers
            qa = io_pool.tile([128, N], f32r, tag="qa", name=f"qa{r}_{p}")
            ka = io_pool.tile([128, N], f32r, tag="ka", name=f"ka{r}_{p}")
            for c in range(4):
                sl = slice(512 * c, 512 * (c + 1))
                nc.gpsimd.dma_start(ka[:, sl], kt_d[p, :, sl])
                nc.gpsimd.dma_start(qa[:, sl], qt_d[p, :, sl])
            va = io_pool.tile([128, MTILES, DA], f16, tag="va", name=f"va{r}_{p}")
            nc.gpsimd.dma_start(va[:], vt_d[p])
            nm = sc_pool.tile([128, 32], f32, tag="nm", name=f"nm{r}_{p}")
            nc.gpsimd.memset(nm[:], 0.0)
            return qa, ka, va, nm

        def snat_phase(st8, i, phase):
            """Half a q-tile's rowmax work: 2 fp32r matmuls over a 1024-col
            m-range into two single-bank PSUM tiles + 2 per-chunk DVE
            reduces.  phase 1 merges the 4 partials into nm[:, i] (negated).
            DVE may read only ONE PSUM operand per instruction, and the
            2-buffer sn ring keeps the PE from ever waiting on a reduce."""
            qa, ka, va, nm = st8
            if phase == 0:
                pm = sc_pool.tile([128, 2], f32, tag="pm", name=f"pm{i}", bufs=4)
                _pm_live[i] = pm
            else:
                pm = _pm_live.pop(i)
            mo = 1024 * phase
            sn = ps_n.tile([128, 1024], f32, tag="sn", name=f"sn{i}_{phase}")
            for c in range(2):
                nc.tensor.matmul(
                    sn[:, 512 * c : 512 * (c + 1)],
                    lhsT=qa[:, 128 * i : 128 * (i + 1)],
                    rhs=ka[:, mo + 512 * c : mo + 512 * (c + 1)],
                    start=True,
                    stop=True,
                )
            nc.vector.tensor_reduce(
                pm[:, phase : phase + 1],
                sn[:],
                axis=mybir.AxisListType.X,
                op=MAX,
            )
            if phase == 1:
                nc.vector.tensor_reduce(
                    nm[:, i : i + 1],
                    pm[:],
                    axis=mybir.AxisListType.X,
                    op=MAX,
                    negate=True,
                )

        _pm_live = {}

        def negmax_row(qa, nm, h):
            """Scatter negmax for q-tiles 8h..8h+7 into qa row 64 (the bias
            row) via a 32x32 block transpose + 4 small strided DMAs (on the
            sync/scalar HWDGE queues -- much lower latency than SWDGE)."""
            nmT = sc_pool.tile([128, 32], f32, tag="nmT", name=f"nmT{h}")
            nc.vector.transpose(nmT[:], nm[:])
            row = qa[64:65, CH * h : CH * (h + 1)].rearrange(
                "p (u j s) -> p u j s", u=8, j=4, s=32
            )
            for j in range(4):
                eng = nc.sync
                eng.dma_start(
                    row[:, :, j],
                    nmT[32 * j + 8 * h : 32 * j + 8 * h + 8, 0:32].bitcast(f32r),
                )

        nxt = load_pair(0)
        # prologue: pair 0, q-tiles 0-7 rowmax.  Phases interleaved across
        # q-tile pairs so the 2-deep sn ring never blocks on a reduce.
        for i in range(0, 8, 2):
            snat_phase(nxt, i, 0)
            snat_phase(nxt, i + 1, 0)
            snat_phase(nxt, i, 1)
            snat_phase(nxt, i + 1, 1)

        for idx in range(len(seq)):
            _, p = seq[idx]
            cur = nxt
            qa, ka, va, nm = cur
            ot_sb = ob_pool.tile([DA, N], f32, tag="ot", name=f"ot{idx}")

            if idx + 1 < len(seq):
                nxt = load_pair(idx + 1)

            for h in range(NH):
                negmax_row(qa, nm, h)
                # front-load three weave phases: bias-independent PE work
                # that covers the bias transpose+DMA latency at the
                # superchunk boundary
                for ph in range(3):
                    if h == 0:
                        snat_phase(cur, 8 + ph // 2, ph % 2)
                    elif idx + 1 < len(seq):
                        snat_phase(nxt, ph // 2, ph % 2)
                pts = [None] * 32
                ovs = [None, None]
                for t in range(35):
                    for side in range(2):
                        # side 0: q cols [0,512); side 1: [512,1024) -- passes
                        # are offset by 16 slots and share one ov PSUM bank
                        tt = t - 16 * side
                        if not (0 <= tt < 16 + 3):
                            continue
                        qo = CH * h + 512 * side
                        if tt == 0:
                            ovs[side] = ps_o.tile(
                                [DA, 512], f32, tag="ov", name=f"ov{idx}_{h}_{side}"
                            )
                        if tt >= 3:
                            m = tt - 3
                            pv = pts[16 * side + m]
                            nc.tensor.matmul(
                                ovs[side][:],
                                lhsT=va[:, m, :],
                                rhs=pv[:],
                                start=(tt == 3),
                                stop=(tt == 18),
                            )
                        if tt < 16:
                            st = ps_s.tile(
                                [128, 512], f32, tag="st", name=f"st{t}_{side}"
                            )
                            nc.tensor.matmul(
                                st[:],
                                lhsT=ka[:, 128 * tt : 128 * (tt + 1)],
                                rhs=qa[:, qo : qo + 512],
                                start=True,
                                stop=True,
                            )
                            pt = pt_pool.tile(
                                [128, 512], f16, tag="pt", name=f"pt{t}_{side}"
                            )
                            nc.scalar.activation(pt[:], st[:], EXP)
                            pts[16 * side + tt] = pt
                        if tt == 18:
                            nc.scalar.copy(
                                ot_sb[:, qo : qo + 512], ovs[side][:]
                            )
                            nc.gpsimd.dma_start(
                                ot_d[p, :, qo : qo + 512],
                                ot_sb[:, qo : qo + 512],
                            )
                    # weave at even slots 0..24: phases 3..15
                    if t % 2 == 0:
                        ph = 3 + t // 2
                        if ph < 16:
                            if h == 0:
                                snat_phase(cur, 8 + ph // 2, ph % 2)
                            elif idx + 1 < len(seq):
                                snat_phase(nxt, ph // 2, ph % 2)

    nc.compile()
    return nc


def _get_nc(reps=1):
    key = f"nc{reps}"
    if key not in _CACHE:
        _CACHE[key] = _build_nc(reps)
    return _CACHE[key]


def _prep_inputs(q, k, v, logit_scale):
    """Host-side marshalling: scale fold, transposes, aug rows, sharding."""
    scale = np.exp(
        np.minimum(logit_scale.astype(np.float32), np.float32(MAX_LOG_SCALE))
    ).reshape(H)
    qs = q.astype(np.float32) * scale[None, :, None, None].astype(np.float32)

    # rows 0-63: data; row 64: bias (q: 0 placeholder / k: ones); 65-127: zero
    qT = np.zeros((B * H, 128, N), np.float32)
    qT[:, :D] = qs.transpose(0, 1, 3, 2).reshape(B * H, D, N)
    kT = np.zeros((B * H, 128, N), np.float32)
    kT[:, :D] = k.astype(np.float32).transpose(0, 1, 3, 2).reshape(B * H, D, N)
    kT[:, D] = 1.0
    # vt[bh, p, t, d] = v[bh, 128 t + p, d], plus a ones column at d=64
    vt = np.ones((B * H, 128, MTILES, DA), np.float16)
    vt[:, :, :, :D] = (
        v.astype(np.float32)
        .reshape(B, H, MTILES, 128, D)
        .transpose(0, 1, 3, 2, 4)
        .reshape(B * H, 128, MTILES, D)
        .astype(np.float16)
    )
    in_maps = []
    for c in range(NCORES):
        sl = slice(PAIRS * c, PAIRS * (c + 1))
        in_maps.append(
            {
                "qt": np.ascontiguousarray(qT[sl]),
                "kt": np.ascontiguousarray(kT[sl]),
                "vt": np.ascontiguousarray(vt[sl]),
            }
        )
    return in_maps


def _assemble(results):
    """out[q, d] = O^T[d, q] / O^T[64, q], transposed back -> [B,H,N,D]."""
    out = np.empty((B * H, N, D), np.float32)
    for c in range(NCORES):
        ot = results[c]["ot"]  # [PAIRS, DA, N]
        for p in range(PAIRS):
            bh = PAIRS * c + p
            out[bh] = (ot[p, :D] / ot[p, D : D + 1]).T
    return out.reshape(B, H, N, D)


def kernel(q, k, v, logit_scale):
    from concourse.bass_utils import run_bass_kernel_spmd

    in_maps = _prep_inputs(q, k, v, logit_scale)
    nc = _get_nc()
    res = run_bass_kernel_spmd(nc, in_maps, list(range(NCORES)))
    return _assemble(res.results)
